# revision 14
# baseline (speedup 1.0000x reference)
"""Causal multi-head attention on 8 trn2 NeuronCores.

Sharding: core = (batch b in {0,1}) x (head-group g in {0..3}; 4 heads each).
QKV weights column-sharded, Wo row-sharded (Megatron TP); each core emits a
partial output for its batch; the host sums the 4 partials per batch and adds
the output bias (the unshard step for row-parallel sharding).

Structure per core: projections and attention are interleaved per sequence
block (proj sc=0, attn qb=0, proj sc=1, ...) so the ACT engine's exp backlog
drains during the PE-dense projection stretches.  Scores for the two heads of
a lockstep pair land in one [128, 2, QB] PSUM tile so a single exp covers
both heads (halves ACT instruction count).  Causality: partial-width score
matmuls, plus a -2000 lower-triangle accumulated via an identity matmul on
diagonal chunks -- exp turns masked entries into exact zeros, so no DVE mask
multiply exists on the critical chain.  v carries a ones column so the AV
matmul produces softmax denominators for free.  Matmul operands are bf16
(PSUM stays fp32; tolerance is 2e-2, bf16 lands ~1e-3).  DMAs are
consolidated (4 for x, 1 per weight, 2 per q-block out) to amortize the
~1.6us/dma_start sequencer issue cost.
"""

import numpy as np
import ml_dtypes

BF = ml_dtypes.bfloat16

B, S, E, H, D = 2, 2048, 1024, 16, 64
NCORES = 8
G = 4            # head-groups (cores per batch)
HPG = H // G     # heads per core = 4
FS = HPG * D     # feature slice per core = 256
P = 128
QB = 512         # query block (matmul moving width)
NQB = S // QB    # 4
NKC = S // P     # 16 k-chunks
EC = E // P      # 8 contraction chunks for projections

_cache = {}


def _split_waits(nc, mybir, max_waits=1):
    """This walrus build encodes at most one sem-wait per instruction.
    Hoist extra waits onto NOPs inserted before the instruction in the same
    engine stream (same basic block => order preserved)."""
    uid = [0]
    for fn in nc.m.functions:
        for bb in fn.blocks:
            new = []
            changed = False
            for inst in bb.instructions:
                si = inst.sync_info
                if si is not None and len(si.on_wait) > max_waits:
                    waits = list(si.on_wait)
                    head, tail = waits[:-max_waits], waits[-max_waits:]
                    for k in range(0, len(head), max_waits):
                        nop = mybir.InstNoOp(name=f"WSPLIT-{uid[0]}", ins=[], outs=[])
                        uid[0] += 1
                        nop.engine = inst.engine
                        nop.sync_info = mybir.SyncInfo(
                            on_wait=head[k:k + max_waits], on_update=[])
                        new.append(nop)
                    inst.sync_info = mybir.SyncInfo(
                        on_wait=tail, on_update=list(si.on_update))
                    changed = True
                new.append(inst)
            if changed:
                bb.instructions = new


def _build(reps=1):
    key = ("nc", reps)
    if key in _cache:
        return _cache[key]
    import os
    no_warm = bool(os.environ.get("ABL_NOWARM"))
    n_warm = int(os.environ.get("N_WARM", "16"))
    bufs_pt = int(os.environ.get("BUFS_PT", "8"))
    bufs_rc = int(os.environ.get("BUFS_RC", "4"))
    bufs_ot = int(os.environ.get("BUFS_OT", "2"))
    bufs_big = int(os.environ.get("BUFS_BIG", "2"))   # score-pair/out-proj PSUM
    bufs_one = int(os.environ.get("BUFS_ONE", "4"))   # av/rb/proj PSUM
    rmode = int(os.environ.get("RECIP_MODE", "0"))    # bit0: approx recip, bit1: direct av*rb
    av_lag = int(os.environ.get("AV_LAG", "2"))

    import concourse.bass as bass
    import concourse.mybir as mybir
    import concourse.tile as tile

    F32 = mybir.dt.float32
    F32R = mybir.dt.float32r
    BF16 = mybir.dt.bfloat16
    EXP = mybir.ActivationFunctionType.Exp

    nc = bass.Bass("TRN2", target_bir_lowering=False, debug=False)

    # x blocks: [P, sc, EC, QB] so one dma_start lands one sc-block
    xb_d = nc.dram_tensor("xb", [P, NQB, EC * QB], BF16, kind="ExternalInput")
    wq_d = nc.dram_tensor("wqt", [P, EC * FS], BF16, kind="ExternalInput")
    wk_d = nc.dram_tensor("wkt", [P, EC * FS], BF16, kind="ExternalInput")
    wv_d = nc.dram_tensor("wvt", [P, EC * FS], BF16, kind="ExternalInput")
    wo_d = nc.dram_tensor("wot", [P, (FS // P) * E], BF16, kind="ExternalInput")
    bq_d = nc.dram_tensor("bq", [P, 2], F32, kind="ExternalInput")
    bk_d = nc.dram_tensor("bk", [P, 2], F32, kind="ExternalInput")
    bv_d = nc.dram_tensor("bvb", [P, FS], F32, kind="ExternalInput")   # pre-broadcast
    # [:, 0:128] = identity; [:, 128:256] = -2000 if j<k else 0 (strict lower)
    msk_d = nc.dram_tensor("msk", [P, 2 * P], BF16, kind="ExternalInput")
    # out: [P, qb, EC, QB]; host reassembles to (E, S)
    out_d = nc.dram_tensor("outb", [P, NQB, EC * QB], BF16, kind="ExternalOutput")

    with tile.TileContext(nc) as tc, \
         nc.allow_low_precision(reason="bf16 matmul operands are intended"), \
         tc.tile_pool(name="big", bufs=1) as big, \
         tc.tile_pool(name="bigd", bufs=2) as bigd, \
         tc.tile_pool(name="small", bufs=1) as small, \
         tc.tile_pool(name="ppA", bufs=bufs_big, space="PSUM") as ppA, \
         tc.tile_pool(name="ppB", bufs=bufs_one, space="PSUM") as ppB, \
         tc.tile_pool(name="pt", bufs=bufs_pt) as ptp, \
         tc.tile_pool(name="rc", bufs=bufs_rc) as rcp, \
         tc.tile_pool(name="ot", bufs=bufs_ot) as otp:
      for _rep in range(reps):
        R = f"r{_rep}"
        if True:

            # ---- resident inputs ----
            # wq/wk first, then x block 0, so QK projections start after
            # ~2.5MB of traffic; weights+small constants ride gpsimd.
            wq_t = big.tile([P, EC * FS], BF16, tag="wq")
            nc.gpsimd.dma_start(wq_t[:], wq_d[:])
            wk_t = big.tile([P, EC * FS], BF16, tag="wk")
            nc.gpsimd.dma_start(wk_t[:], wk_d[:])
            xt = [big.tile([P, EC * QB], BF16, name=f"xt{c}{R}", tag=f"xt{c}")
                  for c in range(NQB)]
            nc.sync.dma_start(xt[0][:], xb_d[:, 0])
            wv_t = big.tile([P, EC * FS], BF16, tag="wv")
            nc.gpsimd.dma_start(wv_t[:], wv_d[:])
            bq_t = small.tile([P, 2], F32, tag="bq")
            nc.gpsimd.dma_start(bq_t[:], bq_d[:])
            bk_t = small.tile([P, 2], F32, tag="bk")
            nc.gpsimd.dma_start(bk_t[:], bk_d[:])
            bv_t = small.tile([P, FS], F32, tag="bv")
            nc.gpsimd.dma_start(bv_t[:], bv_d[:])
            msk_t = small.tile([P, 2 * P], BF16, tag="msk")
            nc.gpsimd.dma_start(msk_t[:], msk_d[:])
            for c in range(1, NQB):
                nc.sync.dma_start(xt[c][:], xb_d[:, c])
            wo_t = bigd.tile([P, 2 * E], BF16, tag="wo")
            nc.gpsimd.dma_start(wo_t[:], wo_d[:])
            ident = msk_t[:, 0:P]
            negtri = msk_t[:, P:2 * P]
            ones_f = small.tile([P, D], F32, tag="onesf")
            nc.any.memset(ones_f[:], 1.0)
            ones_r = small.tile([1, D], F32R, tag="onesr")
            nc.vector.tensor_copy(ones_r[:], ones_f[0:1, :])
            warm_f = small.tile([P, QB], F32, tag="warmf")
            nc.any.memset(warm_f[:], 0.5)
            warm_z = small.tile([P, QB], BF16, tag="warmz")
            nc.vector.tensor_copy(warm_z[:], warm_f[:])

            # ---- resident phase-1 outputs ----
            qT = [bigd.tile([P, S], BF16, name=f"qT{f}{R}", tag=f"qT{f}") for f in range(2)]
            kT = [bigd.tile([P, S], BF16, name=f"kT{f}{R}", tag=f"kT{f}") for f in range(2)]
            vpad = [bigd.tile([P, HPG, D + 1], BF16, name=f"vp{c}{R}", tag=f"vp{c}") for c in range(NKC)]
            attnT = [bigd.tile([P, S], BF16, name=f"aT{f}{R}", tag=f"aT{f}") for f in range(2)]

            # PE warmup: matmuls with no DMA dependency open the HAM clock
            # gate (1.2->2.4GHz) while the input DMAs stream in.
            if not no_warm and _rep == 0:
                wps = ppA.tile([P, QB], F32, tag="big", name=f"wps{R}")
                for wi in range(n_warm):
                    nc.tensor.matmul(wps[:], warm_z[:, 0:P], warm_z[:],
                                     start=(wi == 0), stop=(wi == n_warm - 1))

            def proj(sc):
                for fc in range(2):
                    for dst, w, bias in ((qT, wq_t, bq_t), (kT, wk_t, bk_t)):
                        ps = ppB.tile([P, QB], F32, tag="one")
                        for ec in range(EC):
                            nc.tensor.matmul(
                                ps[:], w[:, ec * FS + fc * P:ec * FS + (fc + 1) * P],
                                xt[sc][:, ec * QB:(ec + 1) * QB],
                                start=(ec == 0), stop=(ec == EC - 1))
                        nc.vector.tensor_add(
                            dst[fc][:, bass.ts(sc, QB)], ps[:],
                            bias[:, fc:fc + 1].to_broadcast((P, QB)))
                for sv in range(4 * sc, 4 * sc + 4):
                    ps = ppB.tile([P, QB], F32, tag="one")
                    psv = ps[:, 0:FS].rearrange("p (h d) -> p h d", h=HPG)
                    sl = (sv - 4 * sc) * P
                    for ec in range(EC):
                        nc.tensor.matmul(
                            ps[:, 0:FS], xt[sc][:, ec * QB + sl:ec * QB + sl + P],
                            wv_t[:, ec * FS:(ec + 1) * FS],
                            start=(ec == 0), stop=(ec == EC - 1))
                    bvv = bv_t.rearrange("p (h d) -> p h d", h=HPG)
                    nc.vector.tensor_add(vpad[sv][:, :, 0:D], psv[:], bvv[:])
                    nc.vector.tensor_copy(vpad[sv][:, :, D:D + 1],
                                          ones_f[:, 0:HPG][:, :, None])

            def attn(qb):
                q0 = qb * QB
                nchunks = (q0 + QB) // P
                hporder = (1, 0) if qb == NQB - 1 else (0, 1)
                for hp in hporder:
                    # two heads in lockstep sharing one score/prob pair tile:
                    # ONE exp instruction covers both heads' scores.
                    heads = (2 * hp, 2 * hp + 1)
                    fc = hp
                    av = {}
                    for i, h in enumerate(heads):
                        av[h] = ppB.tile([D + 1, QB], F32, name=f"av{h}{R}",
                                         tag="one")

                    def av_pair(entry, stop):
                        pT0, d0, c0 = entry
                        for i, h in enumerate(heads):
                            nc.tensor.matmul(
                                av[h][:, d0:QB], vpad[c0][:, h, :],
                                pT0[:, i, d0:QB],
                                start=(c0 == 0), stop=stop)

                    pend = []
                    for c in range(nchunks):
                        delta = max(0, c * P - q0)
                        spp = ppA.tile([P, 2, QB], F32, name=f"spp{hp}{R}",
                                       tag="big")
                        for i, h in enumerate(heads):
                            ro = i * D
                            nc.tensor.matmul(
                                spp[:, i, delta:QB],
                                kT[fc][ro:ro + D, bass.ts(c, P)],
                                qT[fc][ro:ro + D, q0 + delta:q0 + QB],
                                start=True, stop=(c * P < q0))
                            if c * P >= q0:
                                # -2000 strict lower triangle onto the
                                # diagonal square; exp maps it to 0.
                                nc.tensor.matmul(
                                    spp[:, i, delta:delta + P],
                                    ident, negtri,
                                    start=False, stop=True)
                        pT = ptp.tile([P, 2, QB], BF16, name=f"pT{hp}{R}",
                                      tag="pT")
                        nc.scalar.activation(
                            pT[:, :, delta:QB], spp[:, :, delta:QB], EXP,
                            scale=0.125)
                        pend.append((pT, delta, c))
                        # AV lags the scores so exp latency never stalls PE
                        if len(pend) > av_lag:
                            av_pair(pend.pop(0), stop=False)
                    while pend:
                        av_pair(pend.pop(0), stop=(not pend))
                    for i, h in enumerate(heads):
                        ro = i * D
                        if rmode & 1:
                            recip = rcp.tile([1, QB], F32, name=f"rcp{h}{R}",
                                             tag="recip")
                            nc.vector.reciprocal_approx_fast(
                                recip[:], av[h][D:D + 1, :])
                            recip_r = recip[:].bitcast(F32R)
                        else:
                            recip = rcp.tile([1, QB], F32R, name=f"rcp{h}{R}",
                                             tag="recip")
                            nc.vector.reciprocal(recip[:], av[h][D:D + 1, :])
                            recip_r = recip[:]
                        rb = ppB.tile([D, QB], F32, name=f"rb{h}{R}", tag="one")
                        nc.tensor.matmul(rb[:], ones_r[:], recip_r,
                                         start=True, stop=True)
                        if rmode & 2:
                            src_rb = rb[:]
                        else:
                            rbs = rcp.tile([D, QB], F32, name=f"rbs{h}{R}",
                                           tag="rbs")
                            nc.vector.tensor_copy(rbs[:], rb[:])
                            src_rb = rbs[:]
                        nc.vector.tensor_mul(
                            attnT[fc][ro:ro + D, q0:q0 + QB],
                            av[h][0:D, :], src_rb)
            def outproj(qb):
                # first contraction half reads the attnT written by the
                # hp processed first, so the last hp's renorm overlaps it
                fcs = (1, 0) if qb == NQB - 1 else (0, 1)
                ot = otp.tile([P, EC * QB], BF16, tag="ot")
                for m in range(EC):
                    ps = ppA.tile([P, QB], F32, tag="big", name=f"po{R}")
                    for j, fcw in enumerate(fcs):
                        nc.tensor.matmul(ps[:], wo_t[:, fcw * E + m * P:fcw * E + (m + 1) * P],
                                         attnT[fcw][:, bass.ts(qb, QB)],
                                         start=(j == 0), stop=(j == 1))
                    nc.vector.tensor_copy(ot[:, m * QB:(m + 1) * QB], ps[:])
                    if m == EC // 2 - 1:
                        nc.sync.dma_start(out_d[:, qb, 0:EC // 2 * QB],
                                          ot[:, 0:EC // 2 * QB])
                nc.sync.dma_start(out_d[:, qb, EC // 2 * QB:EC * QB],
                                  ot[:, EC // 2 * QB:EC * QB])

            # interleave: P0 A0 P1 O0 A1 P2 O1 A2 P3 O2 A3 O3 -- ACT's exp
            # backlog from attn(qb) drains during proj(qb+1), and the
            # out-projection of qb runs after proj(qb+1) so the last heads'
            # renorm (DVE) never stalls PE.
            proj(0)
            for qb in range(NQB):
                attn(qb)
                if qb + 1 < NQB:
                    proj(qb + 1)
                outproj(qb)

    _split_waits(nc, mybir)
    _cache[key] = nc
    return nc


def _ilv(w):
    """(C*128, N) -> (128, C, N): partition-major interleave for plain DMA."""
    c = w.shape[0] // P
    return np.ascontiguousarray(w.reshape(c, P, w.shape[1]).transpose(1, 0, 2))


def _in_maps(x, Wq, bq, Wk, bk, Wv, bv, Wo, bo):
    f32 = np.float32
    # x[b].T is (E, S); block to [P, NQB, EC, QB]
    xb = [np.ascontiguousarray(
              np.asarray(x[b].T, dtype=f32).reshape(EC, P, NQB, QB)
              .transpose(1, 2, 0, 3)).reshape(P, NQB, EC * QB).astype(BF)
          for b in range(B)]
    WqT = np.ascontiguousarray(Wq.T, dtype=f32)
    WkT = np.ascontiguousarray(Wk.T, dtype=f32)
    WvT = np.ascontiguousarray(Wv.T, dtype=f32)
    # msk: [identity | -2000 strict lower triangle]
    msk = np.zeros((P, 2 * P), dtype=f32)
    msk[:, 0:P] = np.eye(P, dtype=f32)
    msk[:, P:2 * P] = np.tril(np.full((P, P), -2000.0, dtype=f32), k=-1)
    maps = []
    for core in range(NCORES):
        b, g = divmod(core, G)
        fs = slice(g * FS, (g + 1) * FS)
        maps.append({
            "xb": xb[b],
            "wqt": _ilv(WqT[:, fs]).reshape(P, -1).astype(BF),
            "wkt": _ilv(WkT[:, fs]).reshape(P, -1).astype(BF),
            "wvt": _ilv(WvT[:, fs]).reshape(P, -1).astype(BF),
            "wot": _ilv(Wo[:, fs].T).reshape(P, -1).astype(BF),
            "bq": np.ascontiguousarray(bq[fs].reshape(2, P).T),
            "bk": np.ascontiguousarray(bk[fs].reshape(2, P).T),
            "bvb": np.broadcast_to(bv[fs], (P, FS)).copy(),
            "msk": msk.astype(BF),
        })
    return maps


def _runner(reps=1):
    """Compile once; return (exec_fn, put_fn, time_fn).

    put_fn(maps) -> device args (inputs resident on the 8 cores).
    exec_fn(args) -> list of 8 per-core output dicts (numpy).
    """
    rkey = ("run", reps)
    if rkey in _cache:
        return _cache[rkey]

    import jax
    from jax.experimental.shard_map import shard_map
    from jax.sharding import Mesh, NamedSharding, PartitionSpec

    import concourse.mybir as mybir
    from concourse.bass2jax import (
        _bass_exec_p,
        install_neuronx_cc_hook,
        partition_id_tensor,
    )

    nc = _build(reps)
    install_neuronx_cc_hook()

    partition_name = nc.partition_id_tensor.name if nc.partition_id_tensor else None
    in_names, out_names, out_avals, zero_outs = [], [], [], []
    for alloc in nc.m.functions[0].allocations:
        if not isinstance(alloc, mybir.MemoryLocationSet):
            continue
        name = alloc.memorylocations[0].name
        if alloc.kind == "ExternalInput":
            if name != partition_name:
                in_names.append(name)
        elif alloc.kind == "ExternalOutput":
            shape = tuple(alloc.tensor_shape)
            dtype = mybir.dt.np(alloc.dtype)
            out_names.append(name)
            out_avals.append(jax.core.ShapedArray(shape, dtype))
            zero_outs.append(np.zeros(shape, dtype))
    n_params = len(in_names)
    all_in_names = list(in_names) + list(out_names)
    if partition_name is not None:
        all_in_names.append(partition_name)

    def _body(*args):
        operands = list(args)
        if partition_name is not None:
            operands.append(partition_id_tensor())
        outs = _bass_exec_p.bind(
            *operands,
            out_avals=tuple(out_avals),
            in_names=tuple(all_in_names),
            out_names=tuple(out_names),
            lowering_input_output_aliases=(),
            sim_require_finite=True,
            sim_require_nnan=True,
            nc=nc,
        )
        return tuple(outs)

    devices = jax.devices()[:NCORES]
    mesh = Mesh(np.asarray(devices), ("core",))
    n_ops = n_params + len(out_names)
    sharded = jax.jit(
        shard_map(
            _body, mesh=mesh,
            in_specs=(PartitionSpec("core"),) * n_ops,
            out_specs=(PartitionSpec("core"),) * len(out_names),
            check_rep=False,
        ),
        keep_unused=True,
    )
    shard = NamedSharding(mesh, PartitionSpec("core"))

    def put_fn(maps):
        concat = [
            np.concatenate([np.asarray(maps[c][n]) for c in range(NCORES)], axis=0)
            for n in in_names
        ] + [
            np.concatenate([z] * NCORES, axis=0) for z in zero_outs
        ]
        return [jax.device_put(a, shard) for a in concat]

    def exec_fn(args):
        out_arrs = sharded(*args)
        jax.block_until_ready(out_arrs)
        return [
            {
                n: np.asarray(out_arrs[i]).reshape(NCORES, *out_avals[i].shape)[c]
                for i, n in enumerate(out_names)
            }
            for c in range(NCORES)
        ]

    def time_fn(args):
        # device execution only: no output fetch to host
        out_arrs = sharded(*args)
        jax.block_until_ready(out_arrs)

    _cache[rkey] = (exec_fn, put_fn, time_fn)
    return _cache[rkey]


def _assemble(results, bo):
    out = np.empty((B, S, E), dtype=np.float32)
    for b in range(B):
        acc = None
        for g in range(G):
            # [P, NQB, EC, QB] -> (E, S)
            arr = results[b * G + g]["outb"].astype(np.float32)
            arr = arr.reshape(P, NQB, EC, QB)
            full = arr.transpose(2, 0, 1, 3).reshape(E, S)
            acc = full if acc is None else acc + full
        out[b] = acc.T + bo
    return out


def kernel(x, Wq, bq, Wk, bk, Wv, bv, Wo, bo):
    exec_fn, put_fn, _ = _runner()
    maps = _in_maps(x, Wq, bq, Wk, bk, Wv, bv, Wo, bo)
    args = put_fn(maps)
    if not _cache.get("warm"):
        # First execution after load can race device-side initialization;
        # run once and discard, then use the steady-state result.
        exec_fn(args)
        _cache["warm"] = True
    results = exec_fn(args)
    return _assemble(results, bo)


# revision 18
# speedup vs baseline: 1.0623x; 1.0623x over previous
"""Causal multi-head attention on 8 trn2 NeuronCores.

Sharding: core = (batch b in {0,1}) x (head-group g in {0..3}; 4 heads each).
QKV weights column-sharded, Wo row-sharded (Megatron TP); each core emits a
partial output for its batch; the host sums the 4 partials per batch and adds
the output bias (the unshard step for row-parallel sharding).

Structure per core: projections and attention are interleaved per sequence
block (proj sc=0, attn qb=0, proj sc=1, ...) so the ACT engine's exp backlog
drains during the PE-dense projection stretches.  Scores for the two heads of
a lockstep pair land in one [128, 2, QB] PSUM tile so a single exp covers
both heads (halves ACT instruction count).  Causality: partial-width score
matmuls, plus a -2000 lower-triangle accumulated via an identity matmul on
diagonal chunks -- exp turns masked entries into exact zeros, so no DVE mask
multiply exists on the critical chain.  v carries a ones column so the AV
matmul produces softmax denominators for free.  Matmul operands are bf16
(PSUM stays fp32; tolerance is 2e-2, bf16 lands ~1e-3).  DMAs are
consolidated (4 for x, 1 per weight, 2 per q-block out) to amortize the
~1.6us/dma_start sequencer issue cost.
"""

import numpy as np
import ml_dtypes

BF = ml_dtypes.bfloat16

B, S, E, H, D = 2, 2048, 1024, 16, 64
NCORES = 8
G = 4            # head-groups (cores per batch)
HPG = H // G     # heads per core = 4
FS = HPG * D     # feature slice per core = 256
P = 128
QB = 512         # query block (matmul moving width)
NQB = S // QB    # 4
NKC = S // P     # 16 k-chunks
EC = E // P      # 8 contraction chunks for projections

_cache = {}


def _split_waits(nc, mybir, max_waits=1):
    """This walrus build encodes at most one sem-wait per instruction.
    Hoist extra waits onto NOPs inserted before the instruction in the same
    engine stream (same basic block => order preserved)."""
    uid = [0]
    for fn in nc.m.functions:
        for bb in fn.blocks:
            new = []
            changed = False
            for inst in bb.instructions:
                si = inst.sync_info
                if si is not None and len(si.on_wait) > max_waits:
                    waits = list(si.on_wait)
                    head, tail = waits[:-max_waits], waits[-max_waits:]
                    for k in range(0, len(head), max_waits):
                        nop = mybir.InstNoOp(name=f"WSPLIT-{uid[0]}", ins=[], outs=[])
                        uid[0] += 1
                        nop.engine = inst.engine
                        nop.sync_info = mybir.SyncInfo(
                            on_wait=head[k:k + max_waits], on_update=[])
                        new.append(nop)
                    inst.sync_info = mybir.SyncInfo(
                        on_wait=tail, on_update=list(si.on_update))
                    changed = True
                new.append(inst)
            if changed:
                bb.instructions = new


def _build(reps=1):
    key = ("nc", reps)
    if key in _cache:
        return _cache[key]
    import os
    no_warm = bool(os.environ.get("ABL_NOWARM"))
    n_warm = int(os.environ.get("N_WARM", "16"))
    bufs_pt = int(os.environ.get("BUFS_PT", "8"))
    bufs_rc = int(os.environ.get("BUFS_RC", "4"))
    bufs_ot = int(os.environ.get("BUFS_OT", "2"))
    bufs_big = int(os.environ.get("BUFS_BIG", "2"))   # score-pair/out-proj PSUM
    bufs_one = int(os.environ.get("BUFS_ONE", "4"))   # av/rb/proj PSUM
    rmode = int(os.environ.get("RECIP_MODE", "0"))    # bit0: approx recip, bit1: direct av*rb
    av_lag = int(os.environ.get("AV_LAG", "2"))

    import concourse.bass as bass
    import concourse.mybir as mybir
    import concourse.tile as tile

    F32 = mybir.dt.float32
    F32R = mybir.dt.float32r
    BF16 = mybir.dt.bfloat16
    EXP = mybir.ActivationFunctionType.Exp

    nc = bass.Bass("TRN2", target_bir_lowering=False, debug=False)

    # x blocks: [P, sc, EC, QB] so one dma_start lands one sc-block
    xb_d = nc.dram_tensor("xb", [P, NQB, EC * QB], BF16, kind="ExternalInput")
    wq_d = nc.dram_tensor("wqt", [P, EC * FS], BF16, kind="ExternalInput")
    wk_d = nc.dram_tensor("wkt", [P, EC * FS], BF16, kind="ExternalInput")
    wv_d = nc.dram_tensor("wvt", [P, EC * FS], BF16, kind="ExternalInput")
    wo_d = nc.dram_tensor("wot", [P, (FS // P) * E], BF16, kind="ExternalInput")
    bq_d = nc.dram_tensor("bq", [P, 2], F32, kind="ExternalInput")
    bk_d = nc.dram_tensor("bk", [P, 2], F32, kind="ExternalInput")
    bv_d = nc.dram_tensor("bvb", [P, FS], F32, kind="ExternalInput")   # pre-broadcast
    # [:, 0:128] = identity; [:, 128:256] = -2000 if j<k else 0 (strict lower)
    msk_d = nc.dram_tensor("msk", [P, 2 * P], BF16, kind="ExternalInput")
    # out: [P, qb, EC, QB]; host reassembles to (E, S)
    out_d = nc.dram_tensor("outb", [P, NQB, EC * QB], BF16, kind="ExternalOutput")

    with tile.TileContext(nc) as tc, \
         nc.allow_low_precision(reason="bf16 matmul operands are intended"):
      for _rep in range(reps):
        R = f"r{_rep}"
        with tc.tile_pool(name=f"big{R}", bufs=1) as big, \
             tc.tile_pool(name=f"small{R}", bufs=1) as small, \
             tc.tile_pool(name=f"ppA{R}", bufs=bufs_big, space="PSUM") as ppA, \
             tc.tile_pool(name=f"ppB{R}", bufs=bufs_one, space="PSUM") as ppB, \
             tc.tile_pool(name=f"pt{R}", bufs=bufs_pt) as ptp, \
             tc.tile_pool(name=f"rc{R}", bufs=bufs_rc) as rcp, \
             tc.tile_pool(name=f"ot{R}", bufs=bufs_ot) as otp:

            # ---- resident inputs ----
            # wq/wk first, then x block 0, so QK projections start after
            # ~2.5MB of traffic; weights+small constants ride gpsimd.
            wq_t = big.tile([P, EC * FS], BF16, tag="wq")
            nc.gpsimd.dma_start(wq_t[:], wq_d[:])
            wk_t = big.tile([P, EC * FS], BF16, tag="wk")
            nc.gpsimd.dma_start(wk_t[:], wk_d[:])
            xt = [big.tile([P, EC * QB], BF16, name=f"xt{c}{R}", tag=f"xt{c}")
                  for c in range(NQB)]
            nc.sync.dma_start(xt[0][:], xb_d[:, 0])
            wv_t = big.tile([P, EC * FS], BF16, tag="wv")
            nc.gpsimd.dma_start(wv_t[:], wv_d[:])
            bq_t = small.tile([P, 2], F32, tag="bq")
            nc.gpsimd.dma_start(bq_t[:], bq_d[:])
            bk_t = small.tile([P, 2], F32, tag="bk")
            nc.gpsimd.dma_start(bk_t[:], bk_d[:])
            bv_t = small.tile([P, FS], F32, tag="bv")
            nc.gpsimd.dma_start(bv_t[:], bv_d[:])
            msk_t = small.tile([P, 2 * P], BF16, tag="msk")
            nc.gpsimd.dma_start(msk_t[:], msk_d[:])
            for c in range(1, NQB):
                nc.sync.dma_start(xt[c][:], xb_d[:, c])
            wo_t = big.tile([P, 2 * E], BF16, tag="wo")
            nc.gpsimd.dma_start(wo_t[:], wo_d[:])
            tri = msk_t[:, 0:P]
            ones_f = small.tile([P, D], F32, tag="onesf")
            nc.any.memset(ones_f[:], 1.0)
            ones_r = small.tile([1, D], F32R, tag="onesr")
            nc.vector.tensor_copy(ones_r[:], ones_f[0:1, :])
            warm_f = small.tile([P, QB], F32, tag="warmf")
            nc.any.memset(warm_f[:], 0.5)
            warm_z = small.tile([P, QB], BF16, tag="warmz")
            nc.vector.tensor_copy(warm_z[:], warm_f[:])

            # ---- resident phase-1 outputs ----
            qT = [big.tile([P, S], BF16, name=f"qT{f}{R}", tag=f"qT{f}") for f in range(2)]
            kT = [big.tile([P, S], BF16, name=f"kT{f}{R}", tag=f"kT{f}") for f in range(2)]
            vpad = [big.tile([P, HPG, D + 1], BF16, name=f"vp{c}{R}", tag=f"vp{c}") for c in range(NKC)]
            attnT = [big.tile([P, S], BF16, name=f"aT{f}{R}", tag=f"aT{f}") for f in range(2)]

            # PE warmup: matmuls with no DMA dependency open the HAM clock
            # gate (1.2->2.4GHz) while the input DMAs stream in.
            if not no_warm and _rep == 0:
                wps = ppA.tile([P, QB], F32, tag="big", name=f"wps{R}")
                for wi in range(n_warm):
                    nc.tensor.matmul(wps[:], warm_z[:, 0:P], warm_z[:],
                                     start=(wi == 0), stop=(wi == n_warm - 1))

            def proj(sc):
                for fc in range(2):
                    for dst, w, bias in ((qT, wq_t, bq_t), (kT, wk_t, bk_t)):
                        ps = ppB.tile([P, QB], F32, tag="one")
                        for ec in range(EC):
                            nc.tensor.matmul(
                                ps[:], w[:, ec * FS + fc * P:ec * FS + (fc + 1) * P],
                                xt[sc][:, ec * QB:(ec + 1) * QB],
                                start=(ec == 0), stop=(ec == EC - 1))
                        nc.vector.tensor_add(
                            dst[fc][:, bass.ts(sc, QB)], ps[:],
                            bias[:, fc:fc + 1].to_broadcast((P, QB)))
                for sv in range(4 * sc, 4 * sc + 4):
                    ps = ppB.tile([P, QB], F32, tag="one")
                    psv = ps[:, 0:FS].rearrange("p (h d) -> p h d", h=HPG)
                    sl = (sv - 4 * sc) * P
                    for ec in range(EC):
                        nc.tensor.matmul(
                            ps[:, 0:FS], xt[sc][:, ec * QB + sl:ec * QB + sl + P],
                            wv_t[:, ec * FS:(ec + 1) * FS],
                            start=(ec == 0), stop=(ec == EC - 1))
                    bvv = bv_t.rearrange("p (h d) -> p h d", h=HPG)
                    nc.vector.tensor_add(vpad[sv][:, :, 0:D], psv[:], bvv[:])
                    nc.vector.tensor_copy(vpad[sv][:, :, D:D + 1],
                                          ones_f[:, 0:HPG][:, :, None])

            def attn(qb):
                q0 = qb * QB
                nchunks = (q0 + QB) // P
                hporder = (1, 0) if qb == NQB - 1 else (0, 1)
                for hp in hporder:
                    # two heads in lockstep sharing one score/prob pair tile:
                    # ONE exp instruction covers both heads' scores.
                    heads = (2 * hp, 2 * hp + 1)
                    fc = hp
                    av = {}
                    for i, h in enumerate(heads):
                        av[h] = ppB.tile([D + 1, QB], F32, name=f"av{h}{R}",
                                         tag="one")

                    def av_pair(entry, stop):
                        pT0, d0, c0 = entry
                        for i, h in enumerate(heads):
                            nc.tensor.matmul(
                                av[h][:, d0:QB], vpad[c0][:, h, :],
                                pT0[:, i, d0:QB],
                                start=(c0 == 0), stop=stop)

                    pend = []
                    for c in range(nchunks):
                        delta = max(0, c * P - q0)
                        spp = ppA.tile([P, 2, QB], F32, name=f"spp{hp}{R}",
                                       tag="big")
                        for i, h in enumerate(heads):
                            ro = i * D
                            nc.tensor.matmul(
                                spp[:, i, delta:QB],
                                kT[fc][ro:ro + D, bass.ts(c, P)],
                                qT[fc][ro:ro + D, q0 + delta:q0 + QB],
                                start=True, stop=True)
                        pT = ptp.tile([P, 2, QB], BF16, name=f"pT{hp}{R}",
                                      tag="pT")
                        nc.scalar.activation(
                            pT[:, :, delta:QB], spp[:, :, delta:QB], EXP,
                            scale=0.125)
                        if c * P >= q0:
                            # zero the below-diagonal entries of the diagonal
                            # square for both heads in one bf16 2x DVE op;
                            # the AV matmul reads pT two chunks later, so
                            # this sits off the critical chain
                            nc.vector.tensor_mul(
                                pT[:, :, delta:delta + P],
                                pT[:, :, delta:delta + P],
                                tri[:, None, :].to_broadcast((P, 2, P)))
                        pend.append((pT, delta, c))
                        # AV lags the scores so exp latency never stalls PE
                        if len(pend) > av_lag:
                            av_pair(pend.pop(0), stop=False)
                    while pend:
                        av_pair(pend.pop(0), stop=(not pend))
                    for i, h in enumerate(heads):
                        ro = i * D
                        if rmode & 1:
                            recip = rcp.tile([1, QB], F32, name=f"rcp{h}{R}",
                                             tag="recip")
                            nc.vector.reciprocal_approx_fast(
                                recip[:], av[h][D:D + 1, :])
                            recip_r = recip[:].bitcast(F32R)
                        else:
                            recip = rcp.tile([1, QB], F32R, name=f"rcp{h}{R}",
                                             tag="recip")
                            nc.vector.reciprocal(recip[:], av[h][D:D + 1, :])
                            recip_r = recip[:]
                        rb = ppB.tile([D, QB], F32, name=f"rb{h}{R}", tag="one")
                        nc.tensor.matmul(rb[:], ones_r[:], recip_r,
                                         start=True, stop=True)
                        if rmode & 2:
                            src_rb = rb[:]
                        else:
                            rbs = rcp.tile([D, QB], F32, name=f"rbs{h}{R}",
                                           tag="rbs")
                            nc.vector.tensor_copy(rbs[:], rb[:])
                            src_rb = rbs[:]
                        nc.vector.tensor_mul(
                            attnT[fc][ro:ro + D, q0:q0 + QB],
                            av[h][0:D, :], src_rb)
            def outproj(qb):
                # first contraction half reads the attnT written by the
                # hp processed first, so the last hp's renorm overlaps it
                fcs = (1, 0) if qb == NQB - 1 else (0, 1)
                ot = otp.tile([P, EC * QB], BF16, tag="ot")
                for m in range(EC):
                    ps = ppA.tile([P, QB], F32, tag="big", name=f"po{R}")
                    for j, fcw in enumerate(fcs):
                        nc.tensor.matmul(ps[:], wo_t[:, fcw * E + m * P:fcw * E + (m + 1) * P],
                                         attnT[fcw][:, bass.ts(qb, QB)],
                                         start=(j == 0), stop=(j == 1))
                    nc.vector.tensor_copy(ot[:, m * QB:(m + 1) * QB], ps[:])
                    if m == EC // 2 - 1:
                        nc.sync.dma_start(out_d[:, qb, 0:EC // 2 * QB],
                                          ot[:, 0:EC // 2 * QB])
                nc.sync.dma_start(out_d[:, qb, EC // 2 * QB:EC * QB],
                                  ot[:, EC // 2 * QB:EC * QB])

            # interleave: P0 A0 P1 O0 A1 P2 O1 A2 P3 O2 A3 O3 -- ACT's exp
            # backlog from attn(qb) drains during proj(qb+1), and the
            # out-projection of qb runs after proj(qb+1) so the last heads'
            # renorm (DVE) never stalls PE.
            proj(0)
            for qb in range(NQB):
                attn(qb)
                if qb + 1 < NQB:
                    proj(qb + 1)
                outproj(qb)

    _split_waits(nc, mybir)
    _cache[key] = nc
    return nc


def _ilv(w):
    """(C*128, N) -> (128, C, N): partition-major interleave for plain DMA."""
    c = w.shape[0] // P
    return np.ascontiguousarray(w.reshape(c, P, w.shape[1]).transpose(1, 0, 2))


def _in_maps(x, Wq, bq, Wk, bk, Wv, bv, Wo, bo):
    f32 = np.float32
    # x[b].T is (E, S); block to [P, NQB, EC, QB]
    xb = [np.ascontiguousarray(
              np.asarray(x[b].T, dtype=f32).reshape(EC, P, NQB, QB)
              .transpose(1, 2, 0, 3)).reshape(P, NQB, EC * QB).astype(BF)
          for b in range(B)]
    WqT = np.ascontiguousarray(Wq.T, dtype=f32)
    WkT = np.ascontiguousarray(Wk.T, dtype=f32)
    WvT = np.ascontiguousarray(Wv.T, dtype=f32)
    # msk: [causal triangle (1 if j>=k) | unused]
    msk = np.zeros((P, 2 * P), dtype=f32)
    msk[:, 0:P] = np.triu(np.ones((P, P), dtype=f32))
    maps = []
    for core in range(NCORES):
        b, g = divmod(core, G)
        fs = slice(g * FS, (g + 1) * FS)
        maps.append({
            "xb": xb[b],
            "wqt": _ilv(WqT[:, fs]).reshape(P, -1).astype(BF),
            "wkt": _ilv(WkT[:, fs]).reshape(P, -1).astype(BF),
            "wvt": _ilv(WvT[:, fs]).reshape(P, -1).astype(BF),
            "wot": _ilv(Wo[:, fs].T).reshape(P, -1).astype(BF),
            "bq": np.ascontiguousarray(bq[fs].reshape(2, P).T),
            "bk": np.ascontiguousarray(bk[fs].reshape(2, P).T),
            "bvb": np.broadcast_to(bv[fs], (P, FS)).copy(),
            "msk": msk.astype(BF),
        })
    return maps


def _runner(reps=1):
    """Compile once; return (exec_fn, put_fn, time_fn).

    put_fn(maps) -> device args (inputs resident on the 8 cores).
    exec_fn(args) -> list of 8 per-core output dicts (numpy).
    """
    rkey = ("run", reps)
    if rkey in _cache:
        return _cache[rkey]

    import jax
    from jax.experimental.shard_map import shard_map
    from jax.sharding import Mesh, NamedSharding, PartitionSpec

    import concourse.mybir as mybir
    from concourse.bass2jax import (
        _bass_exec_p,
        install_neuronx_cc_hook,
        partition_id_tensor,
    )

    nc = _build(reps)
    install_neuronx_cc_hook()

    partition_name = nc.partition_id_tensor.name if nc.partition_id_tensor else None
    in_names, out_names, out_avals, zero_outs = [], [], [], []
    for alloc in nc.m.functions[0].allocations:
        if not isinstance(alloc, mybir.MemoryLocationSet):
            continue
        name = alloc.memorylocations[0].name
        if alloc.kind == "ExternalInput":
            if name != partition_name:
                in_names.append(name)
        elif alloc.kind == "ExternalOutput":
            shape = tuple(alloc.tensor_shape)
            dtype = mybir.dt.np(alloc.dtype)
            out_names.append(name)
            out_avals.append(jax.core.ShapedArray(shape, dtype))
            zero_outs.append(np.zeros(shape, dtype))
    n_params = len(in_names)
    all_in_names = list(in_names) + list(out_names)
    if partition_name is not None:
        all_in_names.append(partition_name)

    def _body(*args):
        operands = list(args)
        if partition_name is not None:
            operands.append(partition_id_tensor())
        outs = _bass_exec_p.bind(
            *operands,
            out_avals=tuple(out_avals),
            in_names=tuple(all_in_names),
            out_names=tuple(out_names),
            lowering_input_output_aliases=(),
            sim_require_finite=True,
            sim_require_nnan=True,
            nc=nc,
        )
        return tuple(outs)

    devices = jax.devices()[:NCORES]
    mesh = Mesh(np.asarray(devices), ("core",))
    n_ops = n_params + len(out_names)
    sharded = jax.jit(
        shard_map(
            _body, mesh=mesh,
            in_specs=(PartitionSpec("core"),) * n_ops,
            out_specs=(PartitionSpec("core"),) * len(out_names),
            check_rep=False,
        ),
        keep_unused=True,
    )
    shard = NamedSharding(mesh, PartitionSpec("core"))

    def put_fn(maps):
        concat = [
            np.concatenate([np.asarray(maps[c][n]) for c in range(NCORES)], axis=0)
            for n in in_names
        ] + [
            np.concatenate([z] * NCORES, axis=0) for z in zero_outs
        ]
        return [jax.device_put(a, shard) for a in concat]

    def exec_fn(args):
        out_arrs = sharded(*args)
        jax.block_until_ready(out_arrs)
        return [
            {
                n: np.asarray(out_arrs[i]).reshape(NCORES, *out_avals[i].shape)[c]
                for i, n in enumerate(out_names)
            }
            for c in range(NCORES)
        ]

    def time_fn(args):
        # device execution only: no output fetch to host
        out_arrs = sharded(*args)
        jax.block_until_ready(out_arrs)

    _cache[rkey] = (exec_fn, put_fn, time_fn)
    return _cache[rkey]


def _assemble(results, bo):
    out = np.empty((B, S, E), dtype=np.float32)
    for b in range(B):
        acc = None
        for g in range(G):
            # [P, NQB, EC, QB] -> (E, S)
            arr = results[b * G + g]["outb"].astype(np.float32)
            arr = arr.reshape(P, NQB, EC, QB)
            full = arr.transpose(2, 0, 1, 3).reshape(E, S)
            acc = full if acc is None else acc + full
        out[b] = acc.T + bo
    return out


def kernel(x, Wq, bq, Wk, bk, Wv, bv, Wo, bo):
    exec_fn, put_fn, _ = _runner()
    maps = _in_maps(x, Wq, bq, Wk, bk, Wv, bv, Wo, bo)
    args = put_fn(maps)
    if not _cache.get("warm"):
        # First execution after load can race device-side initialization;
        # run once and discard, then use the steady-state result.
        exec_fn(args)
        _cache["warm"] = True
    results = exec_fn(args)
    return _assemble(results, bo)


# revision 19
# speedup vs baseline: 1.2665x; 1.1923x over previous
"""Causal multi-head attention on 8 trn2 NeuronCores.

Sharding: core = (batch b in {0,1}) x (head-group g in {0..3}; 4 heads each).
QKV weights column-sharded, Wo row-sharded (Megatron TP); each core emits a
partial output for its batch; the host sums the 4 partials per batch and adds
the output bias (the unshard step for row-parallel sharding).

Structure per core: projections and attention are interleaved per sequence
block (proj sc=0, attn qb=0, proj sc=1, ...) so the ACT engine's exp backlog
drains during the PE-dense projection stretches.  Scores for the two heads of
a lockstep pair land in one [128, 2, QB] PSUM tile so a single exp covers
both heads (halves ACT instruction count).  Causality: partial-width score
matmuls, plus a -2000 lower-triangle accumulated via an identity matmul on
diagonal chunks -- exp turns masked entries into exact zeros, so no DVE mask
multiply exists on the critical chain.  v carries a ones column so the AV
matmul produces softmax denominators for free.  Matmul operands are bf16
(PSUM stays fp32; tolerance is 2e-2, bf16 lands ~1e-3).  DMAs are
consolidated (4 for x, 1 per weight, 2 per q-block out) to amortize the
~1.6us/dma_start sequencer issue cost.
"""

import numpy as np
import ml_dtypes

BF = ml_dtypes.bfloat16

B, S, E, H, D = 2, 2048, 1024, 16, 64
NCORES = 8
G = 4            # head-groups (cores per batch)
HPG = H // G     # heads per core = 4
FS = HPG * D     # feature slice per core = 256
P = 128
QB = 512         # query block (matmul moving width)
NQB = S // QB    # 4
NKC = S // P     # 16 k-chunks
EC = E // P      # 8 contraction chunks for projections

_cache = {}


def _split_waits(nc, mybir, max_waits=1):
    """This walrus build encodes at most one sem-wait per instruction.
    Hoist extra waits onto NOPs inserted before the instruction in the same
    engine stream (same basic block => order preserved)."""
    uid = [0]
    for fn in nc.m.functions:
        for bb in fn.blocks:
            new = []
            changed = False
            for inst in bb.instructions:
                si = inst.sync_info
                if si is not None and len(si.on_wait) > max_waits:
                    waits = list(si.on_wait)
                    head, tail = waits[:-max_waits], waits[-max_waits:]
                    for k in range(0, len(head), max_waits):
                        nop = mybir.InstNoOp(name=f"WSPLIT-{uid[0]}", ins=[], outs=[])
                        uid[0] += 1
                        nop.engine = inst.engine
                        nop.sync_info = mybir.SyncInfo(
                            on_wait=head[k:k + max_waits], on_update=[])
                        new.append(nop)
                    inst.sync_info = mybir.SyncInfo(
                        on_wait=tail, on_update=list(si.on_update))
                    changed = True
                new.append(inst)
            if changed:
                bb.instructions = new


def _build(reps=1):
    key = ("nc", reps)
    if key in _cache:
        return _cache[key]
    import os
    no_warm = bool(os.environ.get("ABL_NOWARM"))
    n_warm = int(os.environ.get("N_WARM", "16"))
    bufs_pt = int(os.environ.get("BUFS_PT", "8"))
    bufs_rc = int(os.environ.get("BUFS_RC", "4"))
    bufs_ot = int(os.environ.get("BUFS_OT", "2"))
    bufs_big = int(os.environ.get("BUFS_BIG", "2"))   # score-pair/out-proj PSUM
    bufs_one = int(os.environ.get("BUFS_ONE", "4"))   # av/rb/proj PSUM
    rmode = int(os.environ.get("RECIP_MODE", "0"))    # bit0: approx recip, bit1: direct av*rb
    av_lag = int(os.environ.get("AV_LAG", "2"))

    import concourse.bass as bass
    import concourse.mybir as mybir
    import concourse.tile as tile

    F32 = mybir.dt.float32
    F32R = mybir.dt.float32r
    BF16 = mybir.dt.bfloat16
    EXP = mybir.ActivationFunctionType.Exp

    nc = bass.Bass("TRN2", target_bir_lowering=False, debug=False)

    # x blocks: [P, sc, EC, QB] so one dma_start lands one sc-block
    xb_d = nc.dram_tensor("xb", [P, NQB, EC * QB], BF16, kind="ExternalInput")
    wq_d = nc.dram_tensor("wqt", [P, EC * FS], BF16, kind="ExternalInput")
    wk_d = nc.dram_tensor("wkt", [P, EC * FS], BF16, kind="ExternalInput")
    wv_d = nc.dram_tensor("wvt", [P, EC * FS], BF16, kind="ExternalInput")
    wo_d = nc.dram_tensor("wot", [P, (FS // P) * E], BF16, kind="ExternalInput")
    bq_d = nc.dram_tensor("bq", [P, 2], F32, kind="ExternalInput")
    bk_d = nc.dram_tensor("bk", [P, 2], F32, kind="ExternalInput")
    bv_d = nc.dram_tensor("bvb", [P, FS], F32, kind="ExternalInput")   # pre-broadcast
    # [:, 0:128] = identity; [:, 128:256] = -2000 if j<k else 0 (strict lower)
    msk_d = nc.dram_tensor("msk", [P, 2 * P], BF16, kind="ExternalInput")
    # out: [P, qb, EC, QB]; host reassembles to (E, S)
    out_d = nc.dram_tensor("outb", [P, NQB, EC * QB], BF16, kind="ExternalOutput")

    with tile.TileContext(nc) as tc, \
         nc.allow_low_precision(reason="bf16 matmul operands are intended"):
      for _rep in range(reps):
        R = f"r{_rep}"
        with tc.tile_pool(name=f"big{R}", bufs=1) as big, \
             tc.tile_pool(name=f"small{R}", bufs=1) as small, \
             tc.tile_pool(name=f"ppA{R}", bufs=bufs_big, space="PSUM") as ppA, \
             tc.tile_pool(name=f"ppB{R}", bufs=bufs_one, space="PSUM") as ppB, \
             tc.tile_pool(name=f"pt{R}", bufs=bufs_pt) as ptp, \
             tc.tile_pool(name=f"rc{R}", bufs=bufs_rc) as rcp, \
             tc.tile_pool(name=f"ot{R}", bufs=bufs_ot) as otp:

            # ---- resident inputs ----
            # wq/wk first, then x block 0, so QK projections start after
            # ~2.5MB of traffic; weights+small constants ride gpsimd.
            wq_t = big.tile([P, EC * FS], BF16, tag="wq")
            nc.gpsimd.dma_start(wq_t[:], wq_d[:])
            wk_t = big.tile([P, EC * FS], BF16, tag="wk")
            nc.gpsimd.dma_start(wk_t[:], wk_d[:])
            xt = [big.tile([P, EC * QB], BF16, name=f"xt{c}{R}", tag=f"xt{c}")
                  for c in range(NQB)]
            nc.sync.dma_start(xt[0][:], xb_d[:, 0])
            wv_t = big.tile([P, EC * FS], BF16, tag="wv")
            nc.gpsimd.dma_start(wv_t[:], wv_d[:])
            bq_t = small.tile([P, 2], F32, tag="bq")
            nc.gpsimd.dma_start(bq_t[:], bq_d[:])
            bk_t = small.tile([P, 2], F32, tag="bk")
            nc.gpsimd.dma_start(bk_t[:], bk_d[:])
            bv_t = small.tile([P, FS], F32, tag="bv")
            nc.gpsimd.dma_start(bv_t[:], bv_d[:])
            msk_t = small.tile([P, 2 * P], BF16, tag="msk")
            nc.gpsimd.dma_start(msk_t[:], msk_d[:])
            for c in range(1, NQB):
                nc.sync.dma_start(xt[c][:], xb_d[:, c])
            wo_t = big.tile([P, 2 * E], BF16, tag="wo")
            nc.gpsimd.dma_start(wo_t[:], wo_d[:])
            ident = msk_t[:, 0:P]
            negtri = msk_t[:, P:2 * P]
            ones_f = small.tile([P, D], F32, tag="onesf")
            nc.any.memset(ones_f[:], 1.0)
            ones_r = small.tile([1, D], F32R, tag="onesr")
            nc.vector.tensor_copy(ones_r[:], ones_f[0:1, :])
            warm_f = small.tile([P, QB], F32, tag="warmf")
            nc.any.memset(warm_f[:], 0.5)
            warm_z = small.tile([P, QB], BF16, tag="warmz")
            nc.vector.tensor_copy(warm_z[:], warm_f[:])

            # ---- resident phase-1 outputs ----
            qT = [big.tile([P, S], BF16, name=f"qT{f}{R}", tag=f"qT{f}") for f in range(2)]
            kT = [big.tile([P, S], BF16, name=f"kT{f}{R}", tag=f"kT{f}") for f in range(2)]
            vpad = [big.tile([P, HPG, D + 1], BF16, name=f"vp{c}{R}", tag=f"vp{c}") for c in range(NKC)]
            attnT = [big.tile([P, S], BF16, name=f"aT{f}{R}", tag=f"aT{f}") for f in range(2)]

            # PE warmup: matmuls with no DMA dependency open the HAM clock
            # gate (1.2->2.4GHz) while the input DMAs stream in.
            if not no_warm and _rep == 0:
                wps = ppA.tile([P, QB], F32, tag="big", name=f"wps{R}")
                for wi in range(n_warm):
                    nc.tensor.matmul(wps[:], warm_z[:, 0:P], warm_z[:],
                                     start=(wi == 0), stop=(wi == n_warm - 1))

            def proj(sc):
                for fc in range(2):
                    for dst, w, bias in ((qT, wq_t, bq_t), (kT, wk_t, bk_t)):
                        ps = ppB.tile([P, QB], F32, tag="one")
                        for ec in range(EC):
                            nc.tensor.matmul(
                                ps[:], w[:, ec * FS + fc * P:ec * FS + (fc + 1) * P],
                                xt[sc][:, ec * QB:(ec + 1) * QB],
                                start=(ec == 0), stop=(ec == EC - 1))
                        nc.vector.tensor_add(
                            dst[fc][:, bass.ts(sc, QB)], ps[:],
                            bias[:, fc:fc + 1].to_broadcast((P, QB)))
                for sv in range(4 * sc, 4 * sc + 4):
                    ps = ppB.tile([P, QB], F32, tag="one")
                    psv = ps[:, 0:FS].rearrange("p (h d) -> p h d", h=HPG)
                    sl = (sv - 4 * sc) * P
                    for ec in range(EC):
                        nc.tensor.matmul(
                            ps[:, 0:FS], xt[sc][:, ec * QB + sl:ec * QB + sl + P],
                            wv_t[:, ec * FS:(ec + 1) * FS],
                            start=(ec == 0), stop=(ec == EC - 1))
                    bvv = bv_t.rearrange("p (h d) -> p h d", h=HPG)
                    nc.vector.tensor_add(vpad[sv][:, :, 0:D], psv[:], bvv[:])
                    nc.vector.tensor_copy(vpad[sv][:, :, D:D + 1],
                                          ones_f[:, 0:HPG][:, :, None])

            def attn(qb):
                q0 = qb * QB
                nchunks = (q0 + QB) // P
                hporder = (1, 0) if qb == NQB - 1 else (0, 1)
                for hp in hporder:
                    # two heads in lockstep sharing one score/prob pair tile:
                    # ONE exp instruction covers both heads' scores.
                    heads = (2 * hp, 2 * hp + 1)
                    fc = hp
                    av = {}
                    for i, h in enumerate(heads):
                        av[h] = ppB.tile([D + 1, QB], F32, name=f"av{h}{R}",
                                         tag="one")

                    def av_pair(entry, stop):
                        pT0, d0, c0 = entry
                        for i, h in enumerate(heads):
                            nc.tensor.matmul(
                                av[h][:, d0:QB], vpad[c0][:, h, :],
                                pT0[:, i, d0:QB],
                                start=(c0 == 0), stop=stop)

                    pend = []
                    for c in range(nchunks):
                        delta = max(0, c * P - q0)
                        spp = ppA.tile([P, 2, QB], F32, name=f"spp{hp}{R}",
                                       tag="big")
                        for i, h in enumerate(heads):
                            ro = i * D
                            nc.tensor.matmul(
                                spp[:, i, delta:QB],
                                kT[fc][ro:ro + D, bass.ts(c, P)],
                                qT[fc][ro:ro + D, q0 + delta:q0 + QB],
                                start=True, stop=(c * P < q0))
                            if c * P >= q0:
                                # -2000 strict lower triangle onto the
                                # diagonal square; exp maps it to 0.
                                nc.tensor.matmul(
                                    spp[:, i, delta:delta + P],
                                    ident, negtri,
                                    start=False, stop=True)
                        pT = ptp.tile([P, 2, QB], BF16, name=f"pT{hp}{R}",
                                      tag="pT")
                        nc.scalar.activation(
                            pT[:, :, delta:QB], spp[:, :, delta:QB], EXP,
                            scale=0.125)
                        pend.append((pT, delta, c))
                        # AV lags the scores so exp latency never stalls PE
                        if len(pend) > av_lag:
                            av_pair(pend.pop(0), stop=False)
                    while pend:
                        av_pair(pend.pop(0), stop=(not pend))
                    for i, h in enumerate(heads):
                        ro = i * D
                        if rmode & 1:
                            recip = rcp.tile([1, QB], F32, name=f"rcp{h}{R}",
                                             tag="recip")
                            nc.vector.reciprocal_approx_fast(
                                recip[:], av[h][D:D + 1, :])
                            recip_r = recip[:].bitcast(F32R)
                        else:
                            recip = rcp.tile([1, QB], F32R, name=f"rcp{h}{R}",
                                             tag="recip")
                            nc.vector.reciprocal(recip[:], av[h][D:D + 1, :])
                            recip_r = recip[:]
                        rb = ppB.tile([D, QB], F32, name=f"rb{h}{R}", tag="one")
                        nc.tensor.matmul(rb[:], ones_r[:], recip_r,
                                         start=True, stop=True)
                        if rmode & 2:
                            src_rb = rb[:]
                        else:
                            rbs = rcp.tile([D, QB], F32, name=f"rbs{h}{R}",
                                           tag="rbs")
                            nc.vector.tensor_copy(rbs[:], rb[:])
                            src_rb = rbs[:]
                        nc.vector.tensor_mul(
                            attnT[fc][ro:ro + D, q0:q0 + QB],
                            av[h][0:D, :], src_rb)
            def outproj(qb):
                # first contraction half reads the attnT written by the
                # hp processed first, so the last hp's renorm overlaps it
                fcs = (1, 0) if qb == NQB - 1 else (0, 1)
                ot = otp.tile([P, EC * QB], BF16, tag="ot")
                for m in range(EC):
                    ps = ppA.tile([P, QB], F32, tag="big", name=f"po{R}")
                    for j, fcw in enumerate(fcs):
                        nc.tensor.matmul(ps[:], wo_t[:, fcw * E + m * P:fcw * E + (m + 1) * P],
                                         attnT[fcw][:, bass.ts(qb, QB)],
                                         start=(j == 0), stop=(j == 1))
                    nc.vector.tensor_copy(ot[:, m * QB:(m + 1) * QB], ps[:])
                    if m == EC // 2 - 1:
                        nc.sync.dma_start(out_d[:, qb, 0:EC // 2 * QB],
                                          ot[:, 0:EC // 2 * QB])
                nc.sync.dma_start(out_d[:, qb, EC // 2 * QB:EC * QB],
                                  ot[:, EC // 2 * QB:EC * QB])

            # interleave: P0 A0 P1 O0 A1 P2 O1 A2 P3 O2 A3 O3 -- ACT's exp
            # backlog from attn(qb) drains during proj(qb+1), and the
            # out-projection of qb runs after proj(qb+1) so the last heads'
            # renorm (DVE) never stalls PE.
            proj(0)
            for qb in range(NQB):
                attn(qb)
                if qb + 1 < NQB:
                    proj(qb + 1)
                outproj(qb)

    _split_waits(nc, mybir)
    _cache[key] = nc
    return nc


def _ilv(w):
    """(C*128, N) -> (128, C, N): partition-major interleave for plain DMA."""
    c = w.shape[0] // P
    return np.ascontiguousarray(w.reshape(c, P, w.shape[1]).transpose(1, 0, 2))


def _in_maps(x, Wq, bq, Wk, bk, Wv, bv, Wo, bo):
    f32 = np.float32
    # x[b].T is (E, S); block to [P, NQB, EC, QB]
    xb = [np.ascontiguousarray(
              np.asarray(x[b].T, dtype=f32).reshape(EC, P, NQB, QB)
              .transpose(1, 2, 0, 3)).reshape(P, NQB, EC * QB).astype(BF)
          for b in range(B)]
    WqT = np.ascontiguousarray(Wq.T, dtype=f32)
    WkT = np.ascontiguousarray(Wk.T, dtype=f32)
    WvT = np.ascontiguousarray(Wv.T, dtype=f32)
    # msk: [identity | -2000 strict lower triangle]
    msk = np.zeros((P, 2 * P), dtype=f32)
    msk[:, 0:P] = np.eye(P, dtype=f32)
    msk[:, P:2 * P] = np.tril(np.full((P, P), -2000.0, dtype=f32), k=-1)
    maps = []
    for core in range(NCORES):
        b, g = divmod(core, G)
        fs = slice(g * FS, (g + 1) * FS)
        maps.append({
            "xb": xb[b],
            "wqt": _ilv(WqT[:, fs]).reshape(P, -1).astype(BF),
            "wkt": _ilv(WkT[:, fs]).reshape(P, -1).astype(BF),
            "wvt": _ilv(WvT[:, fs]).reshape(P, -1).astype(BF),
            "wot": _ilv(Wo[:, fs].T).reshape(P, -1).astype(BF),
            "bq": np.ascontiguousarray(bq[fs].reshape(2, P).T),
            "bk": np.ascontiguousarray(bk[fs].reshape(2, P).T),
            "bvb": np.broadcast_to(bv[fs], (P, FS)).copy(),
            "msk": msk.astype(BF),
        })
    return maps


def _runner(reps=1):
    """Compile once; return (exec_fn, put_fn, time_fn).

    put_fn(maps) -> device args (inputs resident on the 8 cores).
    exec_fn(args) -> list of 8 per-core output dicts (numpy).
    """
    rkey = ("run", reps)
    if rkey in _cache:
        return _cache[rkey]

    import jax
    from jax.experimental.shard_map import shard_map
    from jax.sharding import Mesh, NamedSharding, PartitionSpec

    import concourse.mybir as mybir
    from concourse.bass2jax import (
        _bass_exec_p,
        install_neuronx_cc_hook,
        partition_id_tensor,
    )

    nc = _build(reps)
    install_neuronx_cc_hook()

    partition_name = nc.partition_id_tensor.name if nc.partition_id_tensor else None
    in_names, out_names, out_avals, zero_outs = [], [], [], []
    for alloc in nc.m.functions[0].allocations:
        if not isinstance(alloc, mybir.MemoryLocationSet):
            continue
        name = alloc.memorylocations[0].name
        if alloc.kind == "ExternalInput":
            if name != partition_name:
                in_names.append(name)
        elif alloc.kind == "ExternalOutput":
            shape = tuple(alloc.tensor_shape)
            dtype = mybir.dt.np(alloc.dtype)
            out_names.append(name)
            out_avals.append(jax.core.ShapedArray(shape, dtype))
            zero_outs.append(np.zeros(shape, dtype))
    n_params = len(in_names)
    all_in_names = list(in_names) + list(out_names)
    if partition_name is not None:
        all_in_names.append(partition_name)

    def _body(*args):
        operands = list(args)
        if partition_name is not None:
            operands.append(partition_id_tensor())
        outs = _bass_exec_p.bind(
            *operands,
            out_avals=tuple(out_avals),
            in_names=tuple(all_in_names),
            out_names=tuple(out_names),
            lowering_input_output_aliases=(),
            sim_require_finite=True,
            sim_require_nnan=True,
            nc=nc,
        )
        return tuple(outs)

    devices = jax.devices()[:NCORES]
    mesh = Mesh(np.asarray(devices), ("core",))
    n_ops = n_params + len(out_names)
    sharded = jax.jit(
        shard_map(
            _body, mesh=mesh,
            in_specs=(PartitionSpec("core"),) * n_ops,
            out_specs=(PartitionSpec("core"),) * len(out_names),
            check_rep=False,
        ),
        keep_unused=True,
    )
    shard = NamedSharding(mesh, PartitionSpec("core"))

    def put_fn(maps):
        concat = [
            np.concatenate([np.asarray(maps[c][n]) for c in range(NCORES)], axis=0)
            for n in in_names
        ] + [
            np.concatenate([z] * NCORES, axis=0) for z in zero_outs
        ]
        return [jax.device_put(a, shard) for a in concat]

    def exec_fn(args):
        out_arrs = sharded(*args)
        jax.block_until_ready(out_arrs)
        return [
            {
                n: np.asarray(out_arrs[i]).reshape(NCORES, *out_avals[i].shape)[c]
                for i, n in enumerate(out_names)
            }
            for c in range(NCORES)
        ]

    def time_fn(args):
        # device execution only: no output fetch to host
        out_arrs = sharded(*args)
        jax.block_until_ready(out_arrs)

    _cache[rkey] = (exec_fn, put_fn, time_fn)
    return _cache[rkey]


def _assemble(results, bo):
    out = np.empty((B, S, E), dtype=np.float32)
    for b in range(B):
        acc = None
        for g in range(G):
            # [P, NQB, EC, QB] -> (E, S)
            arr = results[b * G + g]["outb"].astype(np.float32)
            arr = arr.reshape(P, NQB, EC, QB)
            full = arr.transpose(2, 0, 1, 3).reshape(E, S)
            acc = full if acc is None else acc + full
        out[b] = acc.T + bo
    return out


def kernel(x, Wq, bq, Wk, bk, Wv, bv, Wo, bo):
    exec_fn, put_fn, _ = _runner()
    maps = _in_maps(x, Wq, bq, Wk, bk, Wv, bv, Wo, bo)
    args = put_fn(maps)
    if not _cache.get("warm"):
        # First execution after load can race device-side initialization;
        # run once and discard, then use the steady-state result.
        exec_fn(args)
        _cache["warm"] = True
    results = exec_fn(args)
    return _assemble(results, bo)
